# revision 9
# baseline (speedup 1.0000x reference)
"""Trainium2 Bass kernel for nn_MoELayer_17566416241067 (moe_routing).

Math (single timestep GRU-MoE, PyTorch gate order r,z,n):
  router:  gx_r = x @ W_ih_r.T + b_ih_r ; gh_r = h_r @ W_hh_r.T + b_hh_r
           hr'  = (1-z)*n + z*h_r ; rw = softmax(hr' @ W_fc.T + b_fc)
  experts (batched over e): same GRU cell per expert, then
           eo_e = he'_e @ W_proj_e.T + b_proj_e
           out  = sum_e rw[:, e] * eo_e

Sharding: pure data-parallel over batch across 8 NeuronCores (512 rows each);
weights replicated; no collectives.  All heavy matmuls run transposed
(gate/hidden dim on partitions, batch on the free dim) so per-gate biases fuse
into ScalarE activations.  h_router/h_experts are zeros for this workload
(checked at runtime): the W_hh matmuls collapse into broadcast biases and
h' = (1-z)*n; a general fallback path handles nonzero h.

Precision: gate matmuls in bf16 (intermediates, error squashed by
sigmoid/tanh); projection matmul in float32r (full-rate fp32 storage,
TF32-like multiply) so expert_outputs keep ~1e-4 matmul error.
"""

import numpy as np
import ml_dtypes

import concourse.bass as bass
import concourse.tile as tile
from concourse import bacc, mybir
from concourse.bass_utils import run_bass_kernel_spmd

E = 8
B = 4096
D = 1024
H = 1024
G = 3 * H
N_CORES = 8
BL = B // N_CORES  # 512 batch rows per core
KT = D // 128      # 8 contraction tiles
NT = BL // 128     # 4 batch tiles per core
F32 = mybir.dt.float32
F32R = mybir.dt.float32r
BF16 = mybir.dt.bfloat16
AF = mybir.ActivationFunctionType
BF = ml_dtypes.bfloat16

# Which engine issues DMAs ("sync" = HWDGE, "gpsimd" = SWDGE).
DMA_ENGINE = "gpsimd"


def _dma(nc):
    return getattr(nc, DMA_ENGINE)


def ts(i, size):
    return slice(i * size, (i + 1) * size)


def _build(repeats: int = 1):
    """Fast path (h == 0) per-core Bass module (SPMD: all cores run this)."""
    h_zero = True
    nc = bacc.Bacc(None, target_bir_lowering=False)

    # ---- DRAM I/O (per-core shapes) ----
    xT_d = nc.dram_tensor("xT", [D, BL], BF16, kind="ExternalInput")
    wih_d = nc.dram_tensor("wihT", [E, D, G], BF16, kind="ExternalInput")
    wihr_d = nc.dram_tensor("wihrT", [D, G], BF16, kind="ExternalInput")
    wproj_d = nc.dram_tensor("wprojT", [E, H, D], F32R, kind="ExternalInput")
    bias_e_d = nc.dram_tensor("bias_e", [128, E, 5, KT], F32, kind="ExternalInput")
    bias_r_d = nc.dram_tensor("bias_r", [128, 5, KT], F32, kind="ExternalInput")
    bproj_d = nc.dram_tensor("bproj", [1, 2 * E, 512], BF16, kind="ExternalInput")
    wfc_d = nc.dram_tensor("wfcT", [128, KT, E], BF16, kind="ExternalInput")
    bfc_d = nc.dram_tensor("bfc", [1, E], BF16, kind="ExternalInput")
    out_d = nc.dram_tensor("out", [BL, D], F32, kind="ExternalOutput")
    hrT_d = nc.dram_tensor("hrT", [H, BL], F32, kind="ExternalOutput")
    heT_d = nc.dram_tensor("heT", [E, H, BL], F32, kind="ExternalOutput")
    rw_d = nc.dram_tensor("rw", [BL, E], F32, kind="ExternalOutput")
    eo_d = nc.dram_tensor("eo", [E, BL, D], F32, kind="ExternalOutput")

    with tile.TileContext(nc) as tc:
        with (
            tc.tile_pool(name="const", bufs=1) as const_p,
            tc.tile_pool(name="wih", bufs=9) as wih_p,
            tc.tile_pool(name="wproj", bufs=2) as wproj_p,
            tc.tile_pool(name="he", bufs=2) as he_p,
            tc.tile_pool(name="hr", bufs=1) as hr_p,
            tc.tile_pool(name="acc", bufs=1) as acc_p,
            tc.tile_pool(name="evict", bufs=2) as evict_p,
            tc.tile_pool(name="tmp", bufs=2) as tmp_p,
            tc.tile_pool(name="small", bufs=2) as small_p,
            tc.tile_pool(name="psum", bufs=2, space="PSUM") as psum_p,
        ):
            # ---- constants / persistent tiles ----
            xT_sb = const_p.tile([128, KT, BL], BF16)
            _dma(nc).dma_start(out=xT_sb, in_=xT_d[:].rearrange("(k p) b -> p k b", p=128))
            bias_e_sb = const_p.tile([128, E, 5, KT], F32)
            _dma(nc).dma_start(out=bias_e_sb, in_=bias_e_d[:])
            bias_r_sb = const_p.tile([128, 5, KT], F32)
            _dma(nc).dma_start(out=bias_r_sb, in_=bias_r_d[:])
            wfc_sb = const_p.tile([128, KT, E], BF16)
            _dma(nc).dma_start(out=wfc_sb, in_=wfc_d[:])
            bfc_sb = const_p.tile([1, E], BF16)
            _dma(nc).dma_start(out=bfc_sb, in_=bfc_d[:])
            ones_sb = const_p.tile([1, 128], BF16)
            nc.vector.memset(ones_sb, 1.0)

            def gru_gates(bias_ap, wih_src, he_tile):
                """Transposed GRU cell (h=0): writes he_tile[:, i, :]."""
                slabs = []
                for k in range(KT):
                    s = wih_p.tile([128, G], BF16, tag="wih")
                    _dma(nc).dma_start(out=s, in_=wih_src(k))
                    slabs.append(s)
                for i in range(KT):
                    ps = []
                    for gsec in range(3):
                        g0 = gsec * H + i * 128
                        p = psum_p.tile([128, BL], F32, tag=f"gx{gsec}", bufs=2)
                        for k in range(KT):
                            nc.tensor.matmul(
                                p, lhsT=slabs[k][:, g0:g0 + 128], rhs=xT_sb[:, k, :],
                                start=(k == 0), stop=(k == KT - 1),
                            )
                        ps.append(p)
                    p_r, p_z, p_n = ps
                    # r = sigmoid(gx_r + (b_ih_r + b_hh_r))
                    r_t = tmp_p.tile([128, BL], F32, tag="r")
                    nc.scalar.activation(r_t, p_r, AF.Sigmoid, bias=bias_ap(0, i))
                    # 1-z = sigmoid(-gx_z - (b_ih_z + b_hh_z))  (kind 4 = negated)
                    zc_t = tmp_p.tile([128, BL], F32, tag="zc")
                    nc.scalar.activation(zc_t, p_z, AF.Sigmoid, bias=bias_ap(4, i),
                                         scale=-1.0)
                    # n = tanh(gx_n + b_ih_n + r * b_hh_n)
                    t1 = tmp_p.tile([128, BL], F32, tag="t1")
                    nc.vector.tensor_scalar_mul(t1, r_t, bias_ap(3, i))
                    nc.vector.tensor_add(t1, p_n, t1)
                    n_t = tmp_p.tile([128, BL], F32, tag="n")
                    nc.scalar.activation(n_t, t1, AF.Tanh, bias=bias_ap(2, i))
                    # h' = (1-z) * n
                    nc.vector.tensor_mul(he_tile[:, i, :], n_t, zc_t)

            for _rep in range(repeats):
                # ================= Router =================
                hr_tile = hr_p.tile([128, KT, BL], F32, tag="hr")
                gru_gates(
                    lambda kind, i: bias_r_sb[:, kind, i:i + 1],
                    lambda k: wihr_d[ts(k, 128), :],
                    hr_tile,
                )
                _dma(nc).dma_start(
                    out=hrT_d[:].rearrange("(k p) b -> p k b", p=128), in_=hr_tile
                )
                hr_bf = hr_p.tile([128, KT, BL], BF16, tag="hrbf")
                for k in range(KT):
                    nc.scalar.activation(hr_bf[:, k, :], hr_tile[:, k, :], AF.Copy)

                rw = []
                for t in range(NT):
                    ps_l = psum_p.tile([128, E], F32, tag="eo")
                    for k in range(KT):
                        nc.tensor.matmul(
                            ps_l, lhsT=hr_bf[:, k, ts(t, 128)], rhs=wfc_sb[:, k, :],
                            start=(k == 0), stop=False,
                        )
                    nc.tensor.matmul(ps_l, lhsT=ones_sb, rhs=bfc_sb, start=False, stop=True)
                    ex = small_p.tile([128, E], F32, tag="ex")
                    # logits are small (|l| < ~2): exp is safe without max-shift
                    nc.scalar.activation(ex, ps_l, AF.Exp)
                    sm = small_p.tile([128, 1], F32, tag="sm")
                    nc.vector.reduce_sum(out=sm, in_=ex, axis=mybir.AxisListType.X)
                    ri = small_p.tile([128, 1], F32, tag="ri")
                    nc.vector.reciprocal(ri, sm)
                    rwt = hr_p.tile([128, E], F32, tag=f"rw{t}")
                    nc.vector.tensor_scalar_mul(rwt, ex, ri)
                    _dma(nc).dma_start(out=rw_d[ts(t, 128), :], in_=rwt)
                    rw.append(rwt)

                # ================= Experts =================
                acc = [acc_p.tile([128, D], F32, tag=f"acc{t}", name=f"acc{t}") for t in range(NT)]
                for e in range(E):
                    he_tile = he_p.tile([128, KT, BL], F32R, tag="he")
                    gru_gates(
                        lambda kind, i, e=e: bias_e_sb[:, e, kind, i:i + 1],
                        lambda k, e=e: wih_d[e, ts(k, 128), :],
                        he_tile,
                    )
                    _dma(nc).dma_start(
                        out=heT_d[e].rearrange("(k p) b -> p k b", p=128),
                        in_=he_tile[:].bitcast(F32),
                    )
                    for j in range(2):
                        wp = wproj_p.tile([128, KT, 512], F32R, tag="wp")
                        _dma(nc).dma_start(
                            out=wp,
                            in_=wproj_d[e, :, ts(j, 512)].rearrange("(k p) n -> p k n", p=128),
                        )
                        bpj = small_p.tile([1, 512], BF16, tag="bpj")
                        _dma(nc).dma_start(out=bpj, in_=bproj_d[0:1, 2 * e + j, :])
                        for t in range(NT):
                            ps = psum_p.tile([128, 512], F32, tag="eo")
                            for k in range(KT):
                                nc.tensor.matmul(
                                    ps,
                                    lhsT=he_tile[:, k, ts(t, 128)],
                                    rhs=wp[:, k, :],
                                    start=(k == 0), stop=False,
                                )
                            nc.tensor.matmul(
                                ps, lhsT=ones_sb, rhs=bpj,
                                start=False, stop=True,
                            )
                            eo_sb = evict_p.tile([128, 512], F32, tag="eo_sb")
                            nc.scalar.activation(eo_sb, ps, AF.Copy)
                            _dma(nc).dma_start(
                                out=eo_d[e, ts(t, 128), ts(j, 512)], in_=eo_sb
                            )
                            if e == 0:
                                nc.vector.tensor_scalar_mul(
                                    acc[t][:, ts(j, 512)], eo_sb, rw[t][:, 0:1]
                                )
                            else:
                                wa = tmp_p.tile([128, 512], F32, tag="wacc")
                                nc.vector.tensor_scalar_mul(wa, eo_sb, rw[t][:, e:e + 1])
                                nc.vector.tensor_add(
                                    acc[t][:, ts(j, 512)], acc[t][:, ts(j, 512)], wa
                                )
                for t in range(NT):
                    _dma(nc).dma_start(out=out_d[ts(t, 128), :], in_=acc[t])

    nc.compile()
    return nc


def _build_general(repeats: int = 1):
    """General (h != 0) fallback.  gx and gh accumulate into one PSUM group;
    weights stream as per-gate-tile chunks to fit SBUF.  Perf-secondary."""
    nc = bacc.Bacc(None, target_bir_lowering=False)

    xT_d = nc.dram_tensor("xT", [D, BL], BF16, kind="ExternalInput")
    wih_d = nc.dram_tensor("wihT", [E, D, G], BF16, kind="ExternalInput")
    wihr_d = nc.dram_tensor("wihrT", [D, G], BF16, kind="ExternalInput")
    whh_d = nc.dram_tensor("whhT", [E, H, G], BF16, kind="ExternalInput")
    whhr_d = nc.dram_tensor("whhrT", [H, G], BF16, kind="ExternalInput")
    heT_in_d = nc.dram_tensor("heT_in", [E, H, BL], F32, kind="ExternalInput")
    hrT_in_d = nc.dram_tensor("hrT_in", [H, BL], F32, kind="ExternalInput")
    wproj_d = nc.dram_tensor("wprojT", [E, H, D], F32R, kind="ExternalInput")
    bias_e_d = nc.dram_tensor("bias_e", [128, E, 5, KT], F32, kind="ExternalInput")
    bias_r_d = nc.dram_tensor("bias_r", [128, 5, KT], F32, kind="ExternalInput")
    bproj_d = nc.dram_tensor("bproj", [1, 2 * E, 512], BF16, kind="ExternalInput")
    wfc_d = nc.dram_tensor("wfcT", [128, KT, E], BF16, kind="ExternalInput")
    bfc_d = nc.dram_tensor("bfc", [1, E], BF16, kind="ExternalInput")

    out_d = nc.dram_tensor("out", [BL, D], F32, kind="ExternalOutput")
    hrT_d = nc.dram_tensor("hrT", [H, BL], F32, kind="ExternalOutput")
    heT_d = nc.dram_tensor("heT", [E, H, BL], F32, kind="ExternalOutput")
    rw_d = nc.dram_tensor("rw", [BL, E], F32, kind="ExternalOutput")
    eo_d = nc.dram_tensor("eo", [E, BL, D], F32, kind="ExternalOutput")

    with tile.TileContext(nc) as tc:
        with (
            tc.tile_pool(name="const", bufs=1) as const_p,
            tc.tile_pool(name="wch", bufs=2) as wch_p,
            tc.tile_pool(name="wproj", bufs=2) as wproj_p,
            tc.tile_pool(name="he", bufs=2) as he_p,
            tc.tile_pool(name="hin", bufs=2) as hin_p,
            tc.tile_pool(name="hr", bufs=1) as hr_p,
            tc.tile_pool(name="acc", bufs=1) as acc_p,
            tc.tile_pool(name="evict", bufs=2) as evict_p,
            tc.tile_pool(name="tmp", bufs=2) as tmp_p,
            tc.tile_pool(name="small", bufs=2) as small_p,
            tc.tile_pool(name="psum", bufs=2, space="PSUM") as psum_p,
        ):
            xT_sb = const_p.tile([128, KT, BL], BF16)
            _dma(nc).dma_start(out=xT_sb, in_=xT_d[:].rearrange("(k p) b -> p k b", p=128))
            bias_e_sb = const_p.tile([128, E, 5, KT], F32)
            _dma(nc).dma_start(out=bias_e_sb, in_=bias_e_d[:])
            bias_r_sb = const_p.tile([128, 5, KT], F32)
            _dma(nc).dma_start(out=bias_r_sb, in_=bias_r_d[:])
            wfc_sb = const_p.tile([128, KT, E], BF16)
            _dma(nc).dma_start(out=wfc_sb, in_=wfc_d[:])
            bfc_sb = const_p.tile([1, E], BF16)
            _dma(nc).dma_start(out=bfc_sb, in_=bfc_d[:])
            ones_sb = const_p.tile([1, 128], BF16)
            nc.vector.memset(ones_sb, 1.0)

            def gates2(bias_ap, wih_src, whh_src, hT_f32, hT_bf, he_tile):
                for i in range(KT):
                    gxc, ghc = [], []
                    for k in range(KT):
                        cx = wch_p.tile([128, 3, 128], BF16, tag="wx")
                        ch = wch_p.tile([128, 3, 128], BF16, tag="wh")
                        for sec in range(3):
                            g0 = sec * H + i * 128
                            _dma(nc).dma_start(out=cx[:, sec, :], in_=wih_src(k, g0))
                            _dma(nc).dma_start(out=ch[:, sec, :], in_=whh_src(k, g0))
                        gxc.append(cx)
                        ghc.append(ch)
                    # r, z: gx+gh accumulate into one PSUM group
                    ps = []
                    for gsec in range(2):
                        p = psum_p.tile([128, BL], F32, tag=f"g{gsec}", bufs=1)
                        for k in range(KT):
                            nc.tensor.matmul(
                                p, lhsT=gxc[k][:, gsec, :], rhs=xT_sb[:, k, :],
                                start=(k == 0), stop=False,
                            )
                        for k in range(KT):
                            nc.tensor.matmul(
                                p, lhsT=ghc[k][:, gsec, :], rhs=hT_bf[:, k, :],
                                start=False, stop=(k == KT - 1),
                            )
                        ps.append(p)
                    # n gate: gx_n and gh_n stay separate
                    p_xn = psum_p.tile([128, BL], F32, tag="gxn", bufs=1)
                    for k in range(KT):
                        nc.tensor.matmul(
                            p_xn, lhsT=gxc[k][:, 2, :], rhs=xT_sb[:, k, :],
                            start=(k == 0), stop=(k == KT - 1),
                        )
                    p_hn = psum_p.tile([128, BL], F32, tag="ghn", bufs=1)
                    for k in range(KT):
                        nc.tensor.matmul(
                            p_hn, lhsT=ghc[k][:, 2, :], rhs=hT_bf[:, k, :],
                            start=(k == 0), stop=(k == KT - 1),
                        )
                    p_r, p_z = ps
                    r_t = tmp_p.tile([128, BL], F32, tag="r")
                    nc.scalar.activation(r_t, p_r, AF.Sigmoid, bias=bias_ap(0, i))
                    z_t = tmp_p.tile([128, BL], F32, tag="z")
                    nc.scalar.activation(z_t, p_z, AF.Sigmoid, bias=bias_ap(1, i))
                    # n = tanh(gx_n + b_ih_n + r*(gh_n + b_hh_n))
                    t1 = tmp_p.tile([128, BL], F32, tag="t1")
                    nc.vector.tensor_scalar_add(t1, p_hn, bias_ap(3, i))
                    nc.vector.tensor_mul(t1, r_t, t1)
                    nc.vector.tensor_add(t1, p_xn, t1)
                    n_t = tmp_p.tile([128, BL], F32, tag="n")
                    nc.scalar.activation(n_t, t1, AF.Tanh, bias=bias_ap(2, i))
                    # h' = n - z*n + z*h
                    zn = tmp_p.tile([128, BL], F32, tag="zn")
                    nc.vector.tensor_mul(zn, z_t, n_t)
                    zh = tmp_p.tile([128, BL], F32, tag="zh")
                    nc.vector.tensor_mul(zh, z_t, hT_f32[:, i, :])
                    nc.vector.tensor_sub(zn, n_t, zn)
                    nc.vector.tensor_add(he_tile[:, i, :], zn, zh)

            for _rep in range(repeats):
                # Router
                hrT_in_sb = hin_p.tile([128, KT, BL], F32, tag="hin")
                _dma(nc).dma_start(
                    out=hrT_in_sb, in_=hrT_in_d[:].rearrange("(k p) b -> p k b", p=128)
                )
                hrT_in_bf = hin_p.tile([128, KT, BL], BF16, tag="hinbf")
                for k in range(KT):
                    nc.scalar.activation(hrT_in_bf[:, k, :], hrT_in_sb[:, k, :], AF.Copy)
                hr_tile = hr_p.tile([128, KT, BL], F32, tag="hr")
                gates2(
                    lambda kind, i: bias_r_sb[:, kind, i:i + 1],
                    lambda k, g0: wihr_d[ts(k, 128), g0:g0 + 128],
                    lambda k, g0: whhr_d[ts(k, 128), g0:g0 + 128],
                    hrT_in_sb, hrT_in_bf, hr_tile,
                )
                _dma(nc).dma_start(
                    out=hrT_d[:].rearrange("(k p) b -> p k b", p=128), in_=hr_tile
                )
                hr_bf = hr_p.tile([128, KT, BL], BF16, tag="hrbf")
                for k in range(KT):
                    nc.scalar.activation(hr_bf[:, k, :], hr_tile[:, k, :], AF.Copy)
                rw = []
                for t in range(NT):
                    ps_l = psum_p.tile([128, E], F32, tag="eo", bufs=2)
                    for k in range(KT):
                        nc.tensor.matmul(
                            ps_l, lhsT=hr_bf[:, k, ts(t, 128)], rhs=wfc_sb[:, k, :],
                            start=(k == 0), stop=False,
                        )
                    nc.tensor.matmul(ps_l, lhsT=ones_sb, rhs=bfc_sb, start=False, stop=True)
                    ex = small_p.tile([128, E], F32, tag="ex")
                    nc.scalar.activation(ex, ps_l, AF.Exp)
                    sm = small_p.tile([128, 1], F32, tag="sm")
                    nc.vector.reduce_sum(out=sm, in_=ex, axis=mybir.AxisListType.X)
                    ri = small_p.tile([128, 1], F32, tag="ri")
                    nc.vector.reciprocal(ri, sm)
                    rwt = hr_p.tile([128, E], F32, tag=f"rw{t}", name=f"rw{t}")
                    nc.vector.tensor_scalar_mul(rwt, ex, ri)
                    _dma(nc).dma_start(out=rw_d[ts(t, 128), :], in_=rwt)
                    rw.append(rwt)

                acc = [acc_p.tile([128, D], F32, tag=f"acc{t}", name=f"acc{t}") for t in range(NT)]
                for e in range(E):
                    hT_f32 = hin_p.tile([128, KT, BL], F32, tag="hin")
                    _dma(nc).dma_start(
                        out=hT_f32, in_=heT_in_d[e].rearrange("(k p) b -> p k b", p=128)
                    )
                    hT_bf = hin_p.tile([128, KT, BL], BF16, tag="hinbf")
                    for k in range(KT):
                        nc.scalar.activation(hT_bf[:, k, :], hT_f32[:, k, :], AF.Copy)
                    he_tile = he_p.tile([128, KT, BL], F32R, tag="he")
                    gates2(
                        lambda kind, i, e=e: bias_e_sb[:, e, kind, i:i + 1],
                        lambda k, g0, e=e: wih_d[e, ts(k, 128), g0:g0 + 128],
                        lambda k, g0, e=e: whh_d[e, ts(k, 128), g0:g0 + 128],
                        hT_f32, hT_bf, he_tile,
                    )
                    _dma(nc).dma_start(
                        out=heT_d[e].rearrange("(k p) b -> p k b", p=128),
                        in_=he_tile[:].bitcast(F32),
                    )
                    for j in range(2):
                        wp = wproj_p.tile([128, KT, 512], F32R, tag="wp")
                        _dma(nc).dma_start(
                            out=wp,
                            in_=wproj_d[e, :, ts(j, 512)].rearrange("(k p) n -> p k n", p=128),
                        )
                        bpj = small_p.tile([1, 512], BF16, tag="bpj")
                        _dma(nc).dma_start(out=bpj, in_=bproj_d[0:1, 2 * e + j, :])
                        for t in range(NT):
                            ps = psum_p.tile([128, 512], F32, tag="eo", bufs=2)
                            for k in range(KT):
                                nc.tensor.matmul(
                                    ps,
                                    lhsT=he_tile[:, k, ts(t, 128)],
                                    rhs=wp[:, k, :],
                                    start=(k == 0), stop=False,
                                )
                            nc.tensor.matmul(
                                ps, lhsT=ones_sb, rhs=bpj, start=False, stop=True,
                            )
                            eo_sb = evict_p.tile([128, 512], F32, tag="eo_sb")
                            nc.scalar.activation(eo_sb, ps, AF.Copy)
                            _dma(nc).dma_start(
                                out=eo_d[e, ts(t, 128), ts(j, 512)], in_=eo_sb
                            )
                            if e == 0:
                                nc.vector.tensor_scalar_mul(
                                    acc[t][:, ts(j, 512)], eo_sb, rw[t][:, 0:1]
                                )
                            else:
                                wa = tmp_p.tile([128, 512], F32, tag="wacc")
                                nc.vector.tensor_scalar_mul(wa, eo_sb, rw[t][:, e:e + 1])
                                nc.vector.tensor_add(
                                    acc[t][:, ts(j, 512)], acc[t][:, ts(j, 512)], wa
                                )
                for t in range(NT):
                    _dma(nc).dma_start(out=out_d[ts(t, 128), :], in_=acc[t])

    nc.compile()
    return nc


def _prep_shared(inputs, h_zero):
    f32 = np.float32
    W_ih_e = np.asarray(inputs["W_ih_e"], f32)
    W_hh_e = np.asarray(inputs["W_hh_e"], f32)
    b_ih_e = np.asarray(inputs["b_ih_e"], f32)
    b_hh_e = np.asarray(inputs["b_hh_e"], f32)
    W_proj = np.asarray(inputs["W_proj"], f32)
    b_proj = np.asarray(inputs["b_proj"], f32)
    W_ih_r = np.asarray(inputs["W_ih_r"], f32)
    W_hh_r = np.asarray(inputs["W_hh_r"], f32)
    b_ih_r = np.asarray(inputs["b_ih_r"], f32)
    b_hh_r = np.asarray(inputs["b_hh_r"], f32)
    W_fc = np.asarray(inputs["W_fc"], f32)
    b_fc = np.asarray(inputs["b_fc"], f32)

    def bias_pack(bi, bh):
        # (…, 5, H): 0=r, 1=z, 2=n_ih, 3=n_hh, 4=-z
        k0 = bi[..., :H] + bh[..., :H]
        k1 = bi[..., H:2 * H] + bh[..., H:2 * H]
        k2 = bi[..., 2 * H:]
        k3 = bh[..., 2 * H:]
        return np.stack([k0, k1, k2, k3, -k1], axis=-2)

    be = bias_pack(b_ih_e, b_hh_e)          # (E, 5, H)
    br = bias_pack(b_ih_r, b_hh_r)          # (5, H)
    # (128, E, 5, KT): partition = h % 128, last axis = h // 128
    bias_e = np.ascontiguousarray(
        be.reshape(E, 5, KT, 128).transpose(3, 0, 1, 2)).astype(f32)
    bias_r = np.ascontiguousarray(
        br.reshape(5, KT, 128).transpose(2, 0, 1)).astype(f32)

    shared = {
        "wihT": np.ascontiguousarray(W_ih_e.transpose(0, 2, 1)).astype(BF),
        "wihrT": np.ascontiguousarray(W_ih_r.T).astype(BF),
        "wprojT": np.ascontiguousarray(W_proj.transpose(0, 2, 1)).astype(f32),
        "bias_e": bias_e,
        "bias_r": bias_r,
        "bproj": b_proj.reshape(1, 2 * E, 512).astype(BF),
        "wfcT": np.ascontiguousarray(W_fc.T.reshape(KT, 128, E).transpose(1, 0, 2)).astype(BF),
        "bfc": b_fc.reshape(1, E).astype(BF),
    }
    if not h_zero:
        shared["whhT"] = np.ascontiguousarray(W_hh_e.transpose(0, 2, 1)).astype(BF)
        shared["whhrT"] = np.ascontiguousarray(W_hh_r.T).astype(BF)
    return shared


def _in_maps(inputs, h_zero):
    shared = _prep_shared(inputs, h_zero)
    x = np.asarray(inputs["x"], np.float32)
    maps = []
    for c in range(N_CORES):
        m = dict(shared)
        m["xT"] = np.ascontiguousarray(x[c * BL:(c + 1) * BL, 0, :].T).astype(BF)
        if not h_zero:
            he_in = np.asarray(inputs["h_experts"], np.float32)[:, c * BL:(c + 1) * BL, :]
            hr_in = np.asarray(inputs["h_router"], np.float32)[0, c * BL:(c + 1) * BL, :]
            m["heT_in"] = np.ascontiguousarray(he_in.transpose(0, 2, 1))
            m["hrT_in"] = np.ascontiguousarray(hr_in.T)
        maps.append(m)
    return maps


def _assemble(results):
    out = np.concatenate([r["out"] for r in results], axis=0)                     # (B, D)
    hr = np.concatenate([r["hrT"].T for r in results], axis=0)[None]              # (1, B, H)
    he = np.concatenate([r["heT"].transpose(0, 2, 1) for r in results], axis=1)   # (E, B, H)
    rw = np.concatenate([r["rw"] for r in results], axis=0)                       # (B, E)
    eo = np.concatenate([r["eo"].transpose(1, 0, 2) for r in results], axis=0)    # (B, E, D)
    return (
        np.ascontiguousarray(out, dtype=np.float32),
        np.ascontiguousarray(hr, dtype=np.float32),
        np.ascontiguousarray(he, dtype=np.float32),
        np.ascontiguousarray(rw, dtype=np.float32),
        np.ascontiguousarray(eo, dtype=np.float32),
    )


_CACHE = {}


def _get_module(h_zero, repeats=1):
    key = (h_zero, repeats)
    if key not in _CACHE:
        _CACHE[key] = _build(repeats) if h_zero else _build_general(repeats)
    return _CACHE[key]


def kernel(**inputs):
    h_zero = not (
        np.any(np.asarray(inputs["h_router"])) or np.any(np.asarray(inputs["h_experts"]))
    )
    nc = _get_module(h_zero)
    maps = _in_maps(inputs, h_zero)
    res = run_bass_kernel_spmd(nc, maps, core_ids=list(range(N_CORES)))
    return _assemble(res.results)


if __name__ == "__main__":
    import jax

    key = jax.random.key(0)
    rng = np.random.default_rng(1)
    s = 1.0 / np.sqrt(H)
    u = lambda shp: rng.uniform(-s, s, shp).astype(np.float32)
    inputs = {
        "x": rng.standard_normal((B, 1, D)).astype(np.float32),
        "h_router": np.zeros((1, B, H), np.float32),
        "h_experts": np.zeros((E, B, H), np.float32),
        "W_ih_e": u((E, G, D)), "W_hh_e": u((E, G, H)),
        "b_ih_e": u((E, G)), "b_hh_e": u((E, G)),
        "W_proj": u((E, D, H)), "b_proj": u((E, D)),
        "W_ih_r": u((G, D)), "W_hh_r": u((G, H)),
        "b_ih_r": u((G,)), "b_hh_r": u((G,)),
        "W_fc": u((E, H)), "b_fc": u((E,)),
    }
    outs = kernel(**inputs)
    for i, o in enumerate(outs):
        print(i, o.shape, o.dtype, float(np.abs(o).mean()))
